# revision 1
# baseline (speedup 1.0000x reference)
"""GQA attention block on 8 NeuronCores.

Sharding: tensor-parallel over head groups (4 ways: 8 q heads / 2 kv heads
per core) x data-parallel over batch (2 ways).  Each core computes a partial
y = attn_out_slice @ Wo_slice for its (batch, head-group); the host sums the
4 TP partials per batch element.

Per-core device program (all fp32):
  A) x^T via PE transposes; q^T/k^T/v^T projections (q scaled by 1/sqrt(dh)).
  B) per head: S^T tiles = k^T.T @ q^T, exp on ACT (no max subtraction --
     inputs are scaled gaussians, |S|<~6 so exp is safe in fp32), then
     PV via lhsT=[v|ones]: rows 0..63 accumulate unnormalized out^T, row 64
     accumulates the softmax denominator.  Normalize with a reciprocal +
     partition-broadcast + multiply.
  C) y = out^T.T @ Wo.
"""

import os
import sys

import numpy as np

for _p in ("/opt/trn_rl_repo",):
    if os.path.isdir(_p) and _p not in sys.path:
        sys.path.insert(0, _p)

from contextlib import ExitStack

import concourse.bass as bass  # noqa: F401  (AP types pulled in transitively)
import concourse.mybir as mybir
import concourse.tile as tile
from concourse import bacc
from concourse.bass_utils import run_bass_kernel_spmd
from concourse.masks import make_identity

P = 128
B, T, D = 2, 2048, 2048
HQ, HKV, DH = 32, 8, 64
GROUP = HQ // HKV            # 4
TP = 4                       # tensor-parallel ways
DP = 2                       # data-parallel ways
NCORES = TP * DP
DQ = D // TP                 # 512 q dims per core (8 heads)
DKV = HKV * DH // TP         # 128 kv dims per core (2 kv heads)
NHQ = HQ // TP               # 8 q heads per core
NKV = HKV // TP              # 2 kv heads per core
NKS = D // P                 # 16 contraction subtiles over D
NT = T // P                  # 16 row tiles over T
CH = 512                     # T chunk width in projection phase
NCH = T // CH                # 4
TQB = 512                    # T_q block width in attention / psum bank
NTQB = T // TQB              # 4
NKI = T // P                 # 16 key tiles
NB = D // 512                # 4 output column banks
SCALE = 1.0 / 8.0            # 1/sqrt(DH)
F32 = mybir.dt.float32
F32R = mybir.dt.float32r
AF = mybir.ActivationFunctionType


def _build():
    nc = bacc.Bacc(None, target_bir_lowering=False, debug=False)

    x_ext = nc.dram_tensor("x", [T, D], F32, kind="ExternalInput")
    wq_ext = nc.dram_tensor("wq", [D, DQ], F32, kind="ExternalInput")
    wk_ext = nc.dram_tensor("wk", [D, DKV], F32, kind="ExternalInput")
    wv_ext = nc.dram_tensor("wv", [D, DKV], F32, kind="ExternalInput")
    wo_ext = nc.dram_tensor("wo", [DQ, D], F32, kind="ExternalInput")
    y_ext = nc.dram_tensor("y", [T, D], F32, kind="ExternalOutput")

    x_v = x_ext[:].rearrange("(to p) d -> p to d", p=P)      # [128,16,2048]
    wq_v = wq_ext[:].rearrange("(ko p) m -> p ko m", p=P)    # [128,16,512]
    wk_v = wk_ext[:].rearrange("(ko p) m -> p ko m", p=P)    # [128,16,128]
    wv_v = wv_ext[:].rearrange("(ko p) m -> p ko m", p=P)
    wo_v = wo_ext[:].rearrange("(ko p) n -> p ko n", p=P)    # [128,4,2048]
    y_v = y_ext[:].rearrange("(to p) n -> p to n", p=P)      # [128,16,2048]

    with tile.TileContext(nc) as tc, ExitStack() as ctx:
        const = ctx.enter_context(tc.tile_pool(name="const", bufs=1))
        big = ctx.enter_context(tc.tile_pool(name="big", bufs=3))
        wkv_p = ctx.enter_context(tc.tile_pool(name="wkv", bufs=1))
        row_p = ctx.enter_context(tc.tile_pool(name="rows", bufs=2))
        qt_p = ctx.enter_context(tc.tile_pool(name="qt", bufs=1))
        kt_p = ctx.enter_context(tc.tile_pool(name="kt", bufs=1))
        vo_p = ctx.enter_context(tc.tile_pool(name="vo", bufs=1))
        exp_p = ctx.enter_context(tc.tile_pool(name="expp", bufs=2))
        bc_p = ctx.enter_context(tc.tile_pool(name="bcp", bufs=2))
        rc_p = ctx.enter_context(tc.tile_pool(name="rcp", bufs=2))
        ot_p = ctx.enter_context(tc.tile_pool(name="otp", bufs=1))

        aux_ps = ctx.enter_context(tc.tile_pool(name="aux_ps", bufs=2, space="PSUM"))
        proj_ps = ctx.enter_context(tc.tile_pool(name="proj_ps", bufs=2, space="PSUM"))
        s_ps = ctx.enter_context(tc.tile_pool(name="s_ps", bufs=2, space="PSUM"))
        pv_ps = ctx.enter_context(tc.tile_pool(name="pv_ps", bufs=2, space="PSUM"))

        identity = const.tile([P, P], F32)
        make_identity(nc, identity)
        ones1 = const.tile([1, DH], F32)
        nc.gpsimd.memset(ones1[:], 1.0)

        wq_sb = big.tile([P, NKS, DQ], F32R, tag="big")
        wk_sb = wkv_p.tile([P, NKS, DKV], F32R, tag="wk")
        wv_sb = wkv_p.tile([P, NKS, DKV], F32R, tag="wv")

        qt_sb = qt_p.tile([P, DQ // P, T], F32R)        # q^T * SCALE, [dim, t]
        kt_sb = kt_p.tile([P, T], F32R)                 # k^T, [dim(2 kv heads), t]
        vones = vo_p.tile([P, NKV, NKI, DH + 1], F32R)  # [t%128, kv, t//128, dh|1]
        ones_col = const.tile([P, NKV, NKI], F32)
        nc.gpsimd.memset(ones_col[:], 1.0)
        nc.vector.tensor_copy(vones[:, :, :, DH], ones_col[:])

        # ---- Phase A: x^T chunks + projections ----
        for c in range(NCH):
            xt_ch = big.tile([P, NKS, CH], F32R, tag="big")  # x^T[:, c*CH:+CH]
            for r in range(CH // P):
                xrow = row_p.tile([P, D], F32, tag="rows")
                nc.sync.dma_start(xrow[:], x_v[:, c * (CH // P) + r, :])
                for dsb in range(NKS):
                    tp = aux_ps.tile([P, P], F32, tag="aux")
                    nc.tensor.transpose(tp[:], xrow[:, dsb * P:(dsb + 1) * P], identity)
                    nc.vector.tensor_copy(xt_ch[:, dsb, r * P:(r + 1) * P], tp[:])
            if c == 0:
                # weights go out after the first x rows so PE transposes
                # start as early as possible
                nc.sync.dma_start(wq_sb[:], wq_v.bitcast(F32R))
                nc.sync.dma_start(wk_sb[:], wk_v.bitcast(F32R))
                nc.sync.dma_start(wv_sb[:], wv_v.bitcast(F32R))
            # q^T chunk, scaled by 1/sqrt(dh) on eviction
            for mb in range(DQ // P):
                qp = proj_ps.tile([P, CH], F32, tag="proj")
                for ks in range(NKS):
                    nc.tensor.matmul(
                        qp[:], wq_sb[:, ks, mb * P:(mb + 1) * P],
                        xt_ch[:, ks, :],
                        start=(ks == 0), stop=(ks == NKS - 1))
                nc.scalar.activation(
                    qt_sb[:, mb, c * CH:(c + 1) * CH], qp[:], AF.Copy, scale=SCALE)
            # k^T chunk
            kp = proj_ps.tile([P, CH], F32, tag="proj")
            for ks in range(NKS):
                nc.tensor.matmul(kp[:], wk_sb[:, ks, :],
                                 xt_ch[:, ks, :],
                                 start=(ks == 0), stop=(ks == NKS - 1))
            nc.vector.tensor_copy(kt_sb[:, c * CH:(c + 1) * CH], kp[:])
            # v^T chunk, then PE-transpose into vones (v in natural [t, dh] layout)
            vp = proj_ps.tile([P, CH], F32, tag="proj")
            for ks in range(NKS):
                nc.tensor.matmul(vp[:], wv_sb[:, ks, :],
                                 xt_ch[:, ks, :],
                                 start=(ks == 0), stop=(ks == NKS - 1))
            vt_sb = row_p.tile([P, CH], F32, tag="vt")
            nc.vector.tensor_copy(vt_sb[:], vp[:])
            for r in range(CH // P):
                ki = c * (CH // P) + r
                tp = aux_ps.tile([P, P], F32, tag="aux")
                nc.tensor.transpose(tp[:], vt_sb[:, r * P:(r + 1) * P], identity)
                for j in range(NKV):
                    nc.vector.tensor_copy(vones[:, j, ki, 0:DH],
                                          tp[:, j * DH:(j + 1) * DH])

        # prefetch Wo (slot freed by wq after phase A)
        wo_sb = big.tile([P, DQ // P, D], F32R, tag="big")
        nc.sync.dma_start(wo_sb[:], wo_v.bitcast(F32R))

        # ---- Phases B+C interleaved per T_q block ----
        # q heads are permuted host-side to order [0,4,1,5,2,6,3,7] so that
        # head h sits at (block h%4, partition offset 64*(h//4)) -- the
        # partition offset then always equals its kv head's offset in kt_sb,
        # satisfying matmul's equal-base-partition requirement.
        # out^T is kept per-T_q-block so the Wo matmuls for block tb can
        # overlap the (ACT-bound) attention of block tb+1.
        for tb in range(NTQB):
            outt_tb = ot_p.tile([P, DQ // P, TQB], F32R, tag="ot")
            for h in range(NHQ):
                j = h // GROUP            # kv head on this core
                mbq, poq = h % 4, (h // GROUP) * DH
                pv = pv_ps.tile([DH + 1, TQB], F32, tag="pv")
                for ki in range(NKI):
                    sp = s_ps.tile([P, TQB], F32, tag="s")
                    nc.tensor.matmul(
                        sp[:], kt_sb[j * DH:(j + 1) * DH, ki * P:(ki + 1) * P],
                        qt_sb[poq:poq + DH, mbq, tb * TQB:(tb + 1) * TQB],
                        start=True, stop=True)
                    ex = exp_p.tile([P, TQB], F32R, tag="exp")
                    nc.scalar.activation(ex[:], sp[:], AF.Exp)
                    nc.tensor.matmul(pv[:], vones[:, j, ki, :],
                                     ex[:],
                                     start=(ki == 0), stop=(ki == NKI - 1))
                rc = rc_p.tile([1, TQB], F32, tag="rc")
                nc.vector.reciprocal(rc[:], pv[DH:DH + 1, :])
                bc = bc_p.tile([DH, TQB], F32, tag="bc")
                nc.gpsimd.partition_broadcast(bc[:], rc[:], channels=DH)
                nc.vector.tensor_mul(
                    outt_tb[poq:poq + DH, mbq, :],
                    pv[0:DH, :], bc[:])
            # Wo for the 4 output row-tiles covered by this block
            for mi in range(TQB // P):
                mt = tb * (TQB // P) + mi
                y_sb = row_p.tile([P, D], F32, tag="rows")
                for nb in range(NB):
                    yp = proj_ps.tile([P, 512], F32, tag="proj")
                    for ks in range(DQ // P):
                        nc.tensor.matmul(
                            yp[:], outt_tb[:, ks, mi * P:(mi + 1) * P],
                            wo_sb[:, ks, nb * 512:(nb + 1) * 512],
                            start=(ks == 0), stop=(ks == DQ // P - 1))
                    nc.vector.tensor_copy(y_sb[:, nb * 512:(nb + 1) * 512], yp[:])
                nc.sync.dma_start(y_v[:, mt, :], y_sb[:])

    nc.compile()
    return nc


_NC_CACHE = {}


def _get_nc():
    if "nc" not in _NC_CACHE:
        _NC_CACHE["nc"] = _build()
    return _NC_CACHE["nc"]


def kernel(x, Wq, Wk, Wv, Wo):
    x = np.ascontiguousarray(np.asarray(x, dtype=np.float32))
    Wq = np.asarray(Wq, dtype=np.float32)
    Wk = np.asarray(Wk, dtype=np.float32)
    Wv = np.asarray(Wv, dtype=np.float32)
    Wo = np.asarray(Wo, dtype=np.float32)

    nc = _get_nc()
    # interleave the per-core q heads as [0,4,1,5,2,6,3,7] (see phase B note)
    perm = np.concatenate(
        [np.r_[b * DH:(b + 1) * DH, (b + 4) * DH:(b + 5) * DH] for b in range(4)])
    in_maps = []
    for c in range(NCORES):
        b, g = divmod(c, TP)
        in_maps.append({
            "x": x[b],
            "wq": np.ascontiguousarray(Wq[:, g * DQ:(g + 1) * DQ][:, perm]),
            "wk": np.ascontiguousarray(Wk[:, g * DKV:(g + 1) * DKV]),
            "wv": np.ascontiguousarray(Wv[:, g * DKV:(g + 1) * DKV]),
            "wo": np.ascontiguousarray(Wo[g * DQ:(g + 1) * DQ, :][perm, :]),
        })
    res = run_bass_kernel_spmd(nc, in_maps, list(range(NCORES)))
    y = np.zeros((B, T, D), dtype=np.float32)
    for c in range(NCORES):
        b = c // TP
        y[b] += res.results[c]["y"]
    return y



# revision 15
# speedup vs baseline: 1.8659x; 1.8659x over previous
"""GQA attention block on 8 NeuronCores.

Sharding: tensor-parallel over head groups (4 ways: 8 q heads / 2 kv heads
per core) x data-parallel over batch (2 ways).  Each core computes a partial
y = attn_out_slice @ Wo_slice for its (batch, head-group); the host sums the
4 TP partials per batch element.

v2 design notes (per core, all matmul inputs bf16, PSUM accumulation fp32):
  - Host pre-scales Wq by 1/sqrt(dh) and pre-permutes q heads to
    [0,4,1,5,2,6,3,7] so head h sits at (mb=h%4, partition 64*(h//4)); its
    kv head then lives at the same partition offset in kt, satisfying
    matmul's equal-base-partition rule AND enabling row-tiled S pairs.
  - Phase A: x^T chunks (PE transposes, batched bf16 evictions) + k/v
    projections.  All 4 x^T chunks stay resident so q projections can be
    deferred.
  - Phase B (per 256-col q block): q projection (PE filler that hides under
    exp), then per head-pair (p, p+4): S matmuls use PE row groups 0-1 /
    2-3 (contraction=64) -> issued adjacently they run concurrently; S
    tiles [128,256] are packed 4-per-group into 2 PSUM banks so one ACT
    instruction exps 1024 elems/partition (amortizes ~280cyc/instr
    overhead); PV accumulates [v|1]^T @ ex into a shared pair bank (the
    65th 'ones' row gives the softmax denominator for free).
  - PSUM bank budget (8): pv pairs (1 bank x2) + S groups (2 banks x2) +
    work/aux (1 bank x2).
  - normalization: reciprocal_approx_fast + gpsimd partition broadcast +
    DVE multiply (the plain DVE reciprocal costs 3.3us/instr).
  - NOTE the Tile framework resolves RAW deps in *emission order* only:
    every consumer must be emitted after its producer.  PV's has_written
    gotcha: start=True clears the whole bank, so only the pair's first
    matmul starts the group.
"""

import os
import sys

import numpy as np
import ml_dtypes

for _p in ("/opt/trn_rl_repo",):
    if os.path.isdir(_p) and _p not in sys.path:
        sys.path.insert(0, _p)

from contextlib import ExitStack

import concourse.bass as bass  # noqa: F401  (AP types pulled in transitively)
import concourse.mybir as mybir
import concourse.tile as tile
from concourse import bacc
from concourse.bass_utils import run_bass_kernel_spmd
from concourse.masks import make_identity

P = 128
B, T, D = 2, 2048, 2048
HQ, HKV, DH = 32, 8, 64
GROUP = HQ // HKV            # 4
TP = 4                       # tensor-parallel ways
DP = 2                       # data-parallel ways
NCORES = TP * DP
DQ = D // TP                 # 512 q dims per core (8 heads)
DKV = HKV * DH // TP         # 128 kv dims per core (2 kv heads)
NHQ = HQ // TP               # 8 q heads per core
NKV = HKV // TP              # 2 kv heads per core
NKS = D // P                 # 16 contraction subtiles over D
CH = 512                     # T chunk width in projection phase
NCH = T // CH                # 4
TQ = 256                     # T_q block width in attention
NTQ = T // TQ                # 8
NKI = T // P                 # 16 key tiles
SCALE = 1.0 / 8.0            # 1/sqrt(DH), folded into Wq host-side
F32 = mybir.dt.float32
BF16 = mybir.dt.bfloat16
AF = mybir.ActivationFunctionType
BF = ml_dtypes.bfloat16


def _build():
    nc = bacc.Bacc(None, target_bir_lowering=False, debug=False)

    x_ext = nc.dram_tensor("x", [T, D], BF16, kind="ExternalInput")
    wq_ext = nc.dram_tensor("wq", [D, DQ], BF16, kind="ExternalInput")
    wk_ext = nc.dram_tensor("wk", [D, DKV], BF16, kind="ExternalInput")
    wv_ext = nc.dram_tensor("wv", [D, DKV], BF16, kind="ExternalInput")
    wo_ext = nc.dram_tensor("wo", [DQ, D], BF16, kind="ExternalInput")
    y_ext = nc.dram_tensor("y", [T, D], F32, kind="ExternalOutput")

    x_v = x_ext[:].rearrange("(to p) d -> p to d", p=P)      # [128,16,2048]
    wq_v = wq_ext[:].rearrange("(ko p) m -> p ko m", p=P)    # [128,16,512]
    wk_v = wk_ext[:].rearrange("(ko p) m -> p ko m", p=P)    # [128,16,128]
    wv_v = wv_ext[:].rearrange("(ko p) m -> p ko m", p=P)
    wo_v = wo_ext[:].rearrange("(ko p) n -> p ko n", p=P)    # [128,4,2048]
    y_v = y_ext[:].rearrange("(to p) n -> p to n", p=P)      # [128,16,2048]

    with tile.TileContext(nc) as tc, ExitStack() as ctx:
        const = ctx.enter_context(tc.tile_pool(name="const", bufs=1))
        w_p = ctx.enter_context(tc.tile_pool(name="wp", bufs=1))
        xt_p = ctx.enter_context(tc.tile_pool(name="xtp", bufs=4))
        row_p = ctx.enter_context(tc.tile_pool(name="rows", bufs=2))
        qt_p = ctx.enter_context(tc.tile_pool(name="qt", bufs=1))
        kt_p = ctx.enter_context(tc.tile_pool(name="kt", bufs=1))
        vo_p = ctx.enter_context(tc.tile_pool(name="vo", bufs=1))
        exp_p = ctx.enter_context(tc.tile_pool(name="expp", bufs=4))
        bc_p = ctx.enter_context(tc.tile_pool(name="bcp", bufs=2))
        rc_p = ctx.enter_context(tc.tile_pool(name="rcp", bufs=2))
        ot_p = ctx.enter_context(tc.tile_pool(name="otp", bufs=2))
        y_p = ctx.enter_context(tc.tile_pool(name="yp", bufs=2))

        work_ps = ctx.enter_context(tc.tile_pool(name="work_ps", bufs=2, space="PSUM"))
        s_ps = ctx.enter_context(tc.tile_pool(name="s_ps", bufs=2, space="PSUM"))
        pv_ps = ctx.enter_context(tc.tile_pool(name="pv_ps", bufs=2, space="PSUM"))

        identity = const.tile([P, P], BF16)
        make_identity(nc, identity)

        wq_sb = w_p.tile([P, NKS, DQ], BF16, tag="wq")
        wk_sb = w_p.tile([P, NKS, DKV], BF16, tag="wk")
        wv_sb = w_p.tile([P, NKS, DKV], BF16, tag="wv")
        wo_sb = w_p.tile([P, DQ // P, D], BF16, tag="wo")

        qt_sb = qt_p.tile([P, DQ // P, T], BF16)        # q^T (pre-scaled), [dim, t]
        kt_sb = kt_p.tile([P, T], BF16)                 # k^T, [dim(2 kv heads), t]
        vones = vo_p.tile([P, NKV, NKI, DH + 1], BF16)  # [t%128, kv, t//128, dh|1]
        ones_col = const.tile([P, NKV, NKI], BF16)
        nc.gpsimd.memset(ones_col[:], 1.0)
        nc.vector.tensor_copy(vones[:, :, :, DH], ones_col[:])

        # ---- Phase A: x^T chunks (kept resident) + k/v projections ----
        xt_chunks = []
        for c in range(NCH):
            xt_ch = xt_p.tile([P, NKS, CH], BF16, tag="xt")  # x^T[:, c*CH:+CH]
            xt_chunks.append(xt_ch)
            for r in range(CH // P):
                xrow = row_p.tile([P, D], BF16, tag="rows")
                nc.sync.dma_start(xrow[:], x_v[:, c * (CH // P) + r, :])
                for g in range(2):
                    tp = work_ps.tile([P, 8, P], BF16, tag="w")
                    for t8 in range(8):
                        dsb = g * 8 + t8
                        nc.tensor.transpose(
                            tp[:, t8, :], xrow[:, dsb * P:(dsb + 1) * P], identity)
                    nc.vector.tensor_copy(
                        xt_ch[:, g * 8:(g + 1) * 8, r * P:(r + 1) * P], tp[:])
            if c == 0:
                # weights go out after the first x rows so PE transposes
                # start as early as possible
                nc.sync.dma_start(wq_sb[:], wq_v)
                nc.sync.dma_start(wk_sb[:], wk_v)
                nc.sync.dma_start(wv_sb[:], wv_v)
                nc.sync.dma_start(wo_sb[:], wo_v)
            # k^T chunk
            kp = work_ps.tile([P, CH], F32, tag="w")
            for ks in range(NKS):
                nc.tensor.matmul(kp[:], wk_sb[:, ks, :], xt_ch[:, ks, :],
                                 start=(ks == 0), stop=(ks == NKS - 1))
            nc.vector.tensor_copy(kt_sb[:, c * CH:(c + 1) * CH], kp[:])
            # v^T chunk, then PE-transpose into vones (v in natural [t, dh])
            vp = work_ps.tile([P, CH], F32, tag="w")
            for ks in range(NKS):
                nc.tensor.matmul(vp[:], wv_sb[:, ks, :], xt_ch[:, ks, :],
                                 start=(ks == 0), stop=(ks == NKS - 1))
            vt_sb = row_p.tile([P, CH], BF16, tag="vt")
            nc.vector.tensor_copy(vt_sb[:], vp[:])
            vtp = work_ps.tile([P, CH // P, P], BF16, tag="w")
            for r in range(CH // P):
                nc.tensor.transpose(vtp[:, r, :], vt_sb[:, r * P:(r + 1) * P],
                                    identity)
            for j in range(NKV):
                nc.vector.tensor_copy(
                    vones[:, j, c * (CH // P):(c + 1) * (CH // P), 0:DH],
                    vtp[:, :, j * DH:(j + 1) * DH])

        # ---- Phase B: per T_q block: q proj (exp-hiding PE filler),
        #      attention head-pairs, Wo ----
        def qproj(tb):
            xt_ch = xt_chunks[tb // 2]
            xcols = slice((tb % 2) * TQ, (tb % 2 + 1) * TQ)
            for mb in range(DQ // P):
                qp = work_ps.tile([P, TQ], F32, tag="w")
                for ks in range(NKS):
                    nc.tensor.matmul(
                        qp[:], wq_sb[:, ks, mb * P:(mb + 1) * P],
                        xt_ch[:, ks, xcols],
                        start=(ks == 0), stop=(ks == NKS - 1))
                nc.vector.tensor_copy(qt_sb[:, mb, tb * TQ:(tb + 1) * TQ], qp[:])

        qproj(0)
        for tb in range(NTQ):
            cols = slice(tb * TQ, (tb + 1) * TQ)
            outt = ot_p.tile([P, DQ // P, TQ], BF16, tag="ot")
            for p in range(TP):  # head pair (p, p+4); kv heads (0, 1)
                # both heads share one bank; start=True clears the whole
                # bank's has_written bits so ONLY the pair's first matmul
                # starts -- head B's first then overwrites-on-clear.
                pv = pv_ps.tile([P, NKV, TQ], F32, tag="pv")
                for kg in range(NKI // 2):
                    sg = s_ps.tile([P, 4, TQ], F32, tag="s")
                    ex = exp_p.tile([P, 4, TQ], BF16, tag="exp")
                    for kl in range(2):
                        ki = kg * 2 + kl
                        kc = slice(ki * P, (ki + 1) * P)
                        # bank0 holds A(ki0),A(ki1); bank1 B(ki0),B(ki1)
                        # -> the concurrent (A,B) pair hits 2 banks
                        nc.tensor.matmul(
                            sg[:, kl, :], kt_sb[0:DH, kc],
                            qt_sb[0:DH, p, cols], start=True, stop=True)
                        nc.tensor.matmul(
                            sg[:, 2 + kl, :], kt_sb[DH:2 * DH, kc],
                            qt_sb[DH:2 * DH, p, cols], start=True, stop=True)
                    nc.scalar.activation(ex[:], sg[:], AF.Exp)
                    for kl in range(2):
                        ki = kg * 2 + kl
                        nc.tensor.matmul(
                            pv[0:DH + 1, 0, :], vones[:, 0, ki, :],
                            ex[:, kl, :],
                            start=(ki == 0), stop=(ki == NKI - 1),
                            skip_group_check=True)
                        nc.tensor.matmul(
                            pv[0:DH + 1, 1, :], vones[:, 1, ki, :],
                            ex[:, 2 + kl, :],
                            start=False, stop=(ki == NKI - 1),
                            skip_group_check=True)
                # denominator -> SBUF first: reciprocal_approx_fast's custom
                # DVE microcode mis-reads PSUM operands (verified on HW)
                den = rc_p.tile([1, NKV, TQ], F32, tag="den")
                nc.vector.tensor_copy(den[:], pv[DH:DH + 1, :, :])
                rc = rc_p.tile([1, NKV, TQ], F32, tag="rc")
                nc.vector.reciprocal_approx_fast(rc[:], den[:])
                bcst = bc_p.tile([DH, NKV, TQ], F32, tag="bc")
                nc.gpsimd.partition_broadcast(bcst[:], rc[:], channels=DH)
                nc.vector.tensor_mul(
                    outt[0:DH, p, :], pv[0:DH, 0, :], bcst[:, 0, :])
                nc.vector.tensor_mul(
                    outt[DH:2 * DH, p, :], pv[0:DH, 1, :], bcst[:, 1, :])
            if tb + 1 < NTQ:
                qproj(tb + 1)
            # Wo for the 2 output row-tiles covered by this block
            for mi in range(TQ // P):
                mt = tb * (TQ // P) + mi
                y_sb = y_p.tile([P, D], F32, tag="y")
                for nb in range(4):
                    yp = work_ps.tile([P, 512], F32, tag="w")
                    for ks in range(DQ // P):
                        nc.tensor.matmul(
                            yp[:], outt[:, ks, mi * P:(mi + 1) * P],
                            wo_sb[:, ks, nb * 512:(nb + 1) * 512],
                            start=(ks == 0), stop=(ks == DQ // P - 1))
                    nc.vector.tensor_copy(y_sb[:, nb * 512:(nb + 1) * 512],
                                          yp[:])
                nc.sync.dma_start(y_v[:, mt, :], y_sb[:])

    nc.compile()
    return nc


_NC_CACHE = {}


def _get_nc():
    if "nc" not in _NC_CACHE:
        _NC_CACHE["nc"] = _build()
    return _NC_CACHE["nc"]


# q-head interleave: head h of the core -> (mb h%4, partition 64*(h//4))
_PERM = np.concatenate(
    [np.r_[b * DH:(b + 1) * DH, (b + 4) * DH:(b + 5) * DH] for b in range(4)])


def make_in_maps(x, Wq, Wk, Wv, Wo):
    x = np.asarray(x, dtype=np.float32)
    Wq = np.asarray(Wq, dtype=np.float32) * SCALE
    Wk = np.asarray(Wk, dtype=np.float32)
    Wv = np.asarray(Wv, dtype=np.float32)
    Wo = np.asarray(Wo, dtype=np.float32)
    in_maps = []
    for c in range(NCORES):
        b, g = divmod(c, TP)
        in_maps.append({
            "x": np.ascontiguousarray(x[b]).astype(BF),
            "wq": np.ascontiguousarray(
                Wq[:, g * DQ:(g + 1) * DQ][:, _PERM]).astype(BF),
            "wk": np.ascontiguousarray(Wk[:, g * DKV:(g + 1) * DKV]).astype(BF),
            "wv": np.ascontiguousarray(Wv[:, g * DKV:(g + 1) * DKV]).astype(BF),
            "wo": np.ascontiguousarray(
                Wo[g * DQ:(g + 1) * DQ, :][_PERM, :]).astype(BF),
        })
    return in_maps


def kernel(x, Wq, Wk, Wv, Wo):
    nc = _get_nc()
    in_maps = make_in_maps(x, Wq, Wk, Wv, Wo)
    res = run_bass_kernel_spmd(nc, in_maps, list(range(NCORES)))
    y = np.zeros((B, T, D), dtype=np.float32)
    for c in range(NCORES):
        b = c // TP
        y[b] += res.results[c]["y"]
    return y
